# revision 7
# baseline (speedup 1.0000x reference)
"""Trainium2 Bass kernel: GNN message passing  out = relu((adj @ x) @ W.T + b).

Sharding: 1D row partition of adj across 8 NeuronCores (1024 rows each).

Algebraic refactor: out = relu(adj @ (x @ W.T) + b), with y = x @ W.T computed
exactly on the host (tiny: 8192x512x512), so the device runs a SINGLE matmul.
That matmul runs in fp8e4 DoubleRow mode (2 fp8 MACs/cell/cycle): adj is
centered (B = adj - 0.5) so the uniform[0,1] data sits symmetrically in
e4m3's grid, halving quantization error, and the exact rank-1 term
0.5*colsum(y) is folded into the bias on the host (b' = b + 0.5*colsum(y)),
so the device computes outT = (y8-tiles).T @ B8.T with bias+ReLU fused into
the PSUM->SBUF epilogue and a bf16 store.  Each DoubleRow matmul contracts
256 rows (two 128-row planes along the free dim of both operands).
Verified end-to-end rel err 0.0180 vs the f32 reference (gate 2e-2).
"""

import numpy as np
import ml_dtypes

import concourse.mybir as mybir
from concourse import bacc
from concourse.tile import TileContext
from concourse.bass_utils import run_bass_kernel_spmd

P = 128
N_NODES = 8192
DIM = 512
NCORES = 8
M = N_NODES // NCORES          # 1024 output rows per core
KS = N_NODES // P              # 64 contraction subtiles of 128 rows
KT2 = KS // 2                  # 32 DoubleRow k-tiles (256 rows each)
QT = KT2 // 2                  # 16 double-k2 DMA blocks (512 rows each)
NT = DIM // P                  # 4 tiles of the output-feature dim
FREE = 512                     # PSUM bank width (f32)
MCH = M // FREE                # 2 moving chunks per adj tile row block
BF16 = mybir.dt.bfloat16
F32 = mybir.dt.float32
FP8 = mybir.dt.float8e4
DR = mybir.MatmulPerfMode.DoubleRow

_NC = None


def _build_nc():
    nc = bacc.Bacc("TRN2", debug=False)
    y_d = nc.dram_tensor("y", [N_NODES, DIM], FP8, kind="ExternalInput").ap()
    adjt_d = nc.dram_tensor("adjt", [N_NODES, M], FP8, kind="ExternalInput").ap()
    b_d = nc.dram_tensor("b", [P, NT], F32, kind="ExternalInput").ap()
    out_d = nc.dram_tensor("outt", [DIM, M], BF16, kind="ExternalOutput").ap()

    with TileContext(nc) as tc:
        with (
            tc.tile_pool(name="ysb", bufs=1) as ypool,
            tc.tile_pool(name="bsb", bufs=1) as bpool,
            tc.tile_pool(name="adjr", bufs=2) as adjrpool,
            tc.tile_pool(name="adj", bufs=5) as adjpool,
            tc.tile_pool(name="osb", bufs=4) as opool,
            tc.tile_pool(name="ps", bufs=8, space="PSUM") as pspool,
        ):
            # Stationary operand y8 (32 KB/part), streamed into the k loop so
            # the preload doesn't starve the adj stream.
            y_sb = ypool.tile([P, KS, DIM], FP8)

            ps = [
                [
                    pspool.tile([P, FREE], F32, tag="ps", name=f"ps_{n}_{mc}")
                    for mc in range(MCH)
                ]
                for n in range(NT)
            ]

            def mm_k2(k2, rhs_tile):
                # rhs_tile: [P, 2, M] (pair planes of 128 contraction rows)
                for n in range(NT):
                    for mc in range(MCH):
                        nc.tensor.matmul(
                            ps[n][mc][:],
                            y_sb[:, 2 * k2 : 2 * k2 + 2, n * P : (n + 1) * P],
                            rhs_tile[:, :, mc * FREE : (mc + 1) * FREE],
                            start=(k2 == 0),
                            stop=(k2 == KT2 - 1),
                            perf_mode=DR,
                        )

            # Ramp: first 2 k2 tiles with split descriptors (the first matmul
            # needs only cols 0..511 of both pair planes -> starts after
            # ~128 KB lands).  Issue adj on the sync DGE and y on the scalar
            # DGE so the two streams don't serialize on one sequencer.
            RAMP_K2 = 2
            for k2 in range(RAMP_K2):
                nc.scalar.dma_start(
                    y_sb[:, 2 * k2 : 2 * k2 + 2, :],
                    y_d[k2 * 2 * P : (k2 + 1) * 2 * P, :].rearrange(
                        "(two p) n -> p two n", p=P
                    ),
                )
                adj_sb = adjrpool.tile([P, 2, M], FP8, tag="adjr", name=f"adjr_{k2}")
                for mc in range(MCH):
                    nc.sync.dma_start(
                        adj_sb[:, :, mc * FREE : (mc + 1) * FREE],
                        adjt_d[
                            k2 * 2 * P : (k2 + 1) * 2 * P,
                            mc * FREE : (mc + 1) * FREE,
                        ].rearrange("(two p) m -> p two m", p=P),
                    )
                mm_k2(k2, adj_sb)

            b_sb = bpool.tile([P, NT], F32)
            nc.scalar.dma_start(b_sb[:], b_d[:])

            # Steady phase: 2 k2 tiles (512 contraction rows) per descriptor
            # for both operands; adj alternates between the sync and scalar
            # DGEs to halve per-sequencer issue load and deepen prefetch.
            for q in range(RAMP_K2 // 2, QT):
                nc.scalar.dma_start(
                    y_sb[:, 4 * q : 4 * q + 4, :],
                    y_d[q * 4 * P : (q + 1) * 4 * P, :].rearrange(
                        "(four p) n -> p four n", p=P
                    ),
                )
                adj2_sb = adjpool.tile(
                    [P, 2, 2, M], FP8, tag="adj", name=f"adj_{q}"
                )
                eng = nc.sync if q % 2 == 0 else nc.scalar
                eng.dma_start(
                    adj2_sb[:],
                    adjt_d[q * 4 * P : (q + 1) * 4 * P, :].rearrange(
                        "(kk two p) m -> p kk two m", p=P, two=2
                    ),
                )
                for kk in range(2):
                    mm_k2(2 * q + kk, adj2_sb[:, kk])

            # Epilogue: bias+ReLU straight out of PSUM (ACT and DVE alternate
            # across chunks so the tail isn't serialized on one engine), bf16
            # store, one [128, 512] descriptor per (n, mc) chunk so the last
            # store starts as soon as its own chunk is done.
            for n in range(NT):
                pair_sb = opool.tile([P, 2 * FREE], BF16, tag="osb", name=f"o_{n}")
                for mc in range(MCH):
                    dst = pair_sb[:, mc * FREE : (mc + 1) * FREE]
                    if (2 * n + mc) % 2 == 0:
                        nc.scalar.activation(
                            dst,
                            ps[n][mc][:],
                            mybir.ActivationFunctionType.Relu,
                            bias=b_sb[:, n : n + 1],
                        )
                    else:
                        nc.vector.tensor_scalar(
                            dst,
                            ps[n][mc][:],
                            b_sb[:, n : n + 1],
                            0.0,
                            mybir.AluOpType.add,
                            mybir.AluOpType.max,
                        )
                    eng = nc.sync if mc == 0 else nc.scalar
                    eng.dma_start(
                        out_d[n * P : (n + 1) * P, mc * FREE : (mc + 1) * FREE],
                        pair_sb[:, mc * FREE : (mc + 1) * FREE],
                    )
    nc.finalize()
    return nc


def _get_nc():
    global _NC
    if _NC is None:
        _NC = _build_nc()
    return _NC


def _prepare(inputs):
    f8 = ml_dtypes.float8_e4m3
    x = np.asarray(inputs["x"], dtype=np.float32)
    adj = np.asarray(inputs["adj"], dtype=np.float32)
    W = np.asarray(inputs["W"], dtype=np.float32)
    b = np.asarray(inputs["b"], dtype=np.float32)

    # Host-side algebra: y = x @ W.T exactly; fold adj's mean row into the
    # bias:  adj @ y = (B + 0.5) @ y  =>  b' = b + 0.5 * colsum(y).
    y = x @ W.T
    y8 = np.ascontiguousarray(y.astype(f8))
    bprime = (
        b.astype(np.float64) + 0.5 * y.sum(axis=0, dtype=np.float64)
    ).astype(np.float32)
    b_tiled = np.ascontiguousarray(bprime.reshape(NT, P).T)  # [128, 4]

    b8t = (adj - 0.5).astype(f8).T  # [K, rows] view

    in_maps = []
    for c in range(NCORES):
        in_maps.append(
            {
                "y": y8,
                "adjt": np.ascontiguousarray(b8t[:, c * M : (c + 1) * M]),
                "b": b_tiled,
            }
        )
    return in_maps


def _run(in_maps, **kwargs):
    return run_bass_kernel_spmd(
        _get_nc(), in_maps, core_ids=list(range(NCORES)), **kwargs
    )


def _assemble(results):
    out = np.empty((N_NODES, DIM), dtype=np.float32)
    for c in range(NCORES):
        out[c * M : (c + 1) * M, :] = results[c]["outt"].astype(np.float32).T
    return out


def kernel(**inputs):
    res = _run(_prepare(inputs))
    return _assemble(res.results)


# revision 9
# speedup vs baseline: 1.0959x; 1.0959x over previous
"""Trainium2 Bass kernel: GNN message passing  out = relu((adj @ x) @ W.T + b).

Sharding: 1D row partition of adj across 8 NeuronCores (1024 rows each).

Algebraic refactor: out = relu(adj @ (x @ W.T) + b), with y = x @ W.T computed
exactly on the host (tiny: 8192x512x512), so the device runs a SINGLE matmul.
That matmul runs in fp8e4 DoubleRow mode (2 fp8 MACs/cell/cycle): adj is
centered (B = adj - 0.5) so the uniform[0,1] data sits symmetrically in
e4m3's grid, halving quantization error, and the exact rank-1 term
0.5*colsum(y) is folded into the bias on the host (b' = b + 0.5*colsum(y)),
so the device computes outT = (y8-tiles).T @ B8.T with bias+ReLU fused into
the PSUM->SBUF epilogue and a bf16 store.  Each DoubleRow matmul contracts
256 rows (two 128-row planes along the free dim of both operands).
Verified end-to-end rel err 0.0180 vs the f32 reference (gate 2e-2).
"""

import numpy as np
import ml_dtypes

import concourse.mybir as mybir
from concourse import bacc
from concourse.tile import TileContext
from concourse.bass_utils import run_bass_kernel_spmd

P = 128
N_NODES = 8192
DIM = 512
NCORES = 8
M = N_NODES // NCORES          # 1024 output rows per core
KS = N_NODES // P              # 64 contraction subtiles of 128 rows
KT2 = KS // 2                  # 32 DoubleRow k-tiles (256 rows each)
QT = KT2 // 2                  # 16 double-k2 DMA blocks (512 rows each)
NT = DIM // P                  # 4 tiles of the output-feature dim
FREE = 512                     # PSUM bank width (f32)
MCH = M // FREE                # 2 moving chunks per adj tile row block
BF16 = mybir.dt.bfloat16
F32 = mybir.dt.float32
FP8 = mybir.dt.float8e4
DR = mybir.MatmulPerfMode.DoubleRow

_NC = None


def _build_nc():
    nc = bacc.Bacc("TRN2", debug=False)
    y_d = nc.dram_tensor("y", [N_NODES, DIM], FP8, kind="ExternalInput").ap()
    adjt_d = nc.dram_tensor("adjt", [N_NODES, M], FP8, kind="ExternalInput").ap()
    b_d = nc.dram_tensor("b", [P, NT], F32, kind="ExternalInput").ap()
    out_d = nc.dram_tensor("outt", [DIM, M], BF16, kind="ExternalOutput").ap()

    with TileContext(nc) as tc:
        with (
            tc.tile_pool(name="ysb", bufs=1) as ypool,
            tc.tile_pool(name="bsb", bufs=1) as bpool,
            tc.tile_pool(name="adjr", bufs=2) as adjrpool,
            tc.tile_pool(name="adj", bufs=5) as adjpool,
            tc.tile_pool(name="osb", bufs=4) as opool,
            tc.tile_pool(name="ps", bufs=8, space="PSUM") as pspool,
        ):
            # Stationary operand y8 (32 KB/part), streamed into the k loop so
            # the preload doesn't starve the adj stream.
            y_sb = ypool.tile([P, KS, DIM], FP8)

            ps = [
                [
                    pspool.tile([P, FREE], F32, tag="ps", name=f"ps_{n}_{mc}")
                    for mc in range(MCH)
                ]
                for n in range(NT)
            ]

            def mm_k2(k2, rhs_tile):
                # rhs_tile: [P, 2, M] (pair planes of 128 contraction rows)
                for n in range(NT):
                    for mc in range(MCH):
                        nc.tensor.matmul(
                            ps[n][mc][:],
                            y_sb[:, 2 * k2 : 2 * k2 + 2, n * P : (n + 1) * P],
                            rhs_tile[:, :, mc * FREE : (mc + 1) * FREE],
                            start=(k2 == 0),
                            stop=(k2 == KT2 - 1),
                            perf_mode=DR,
                        )

            # Ramp: first 2 k2 tiles with split descriptors (the first matmul
            # needs only cols 0..511 of both pair planes -> starts after
            # ~128 KB lands).  Issue adj on the sync DGE and y on the scalar
            # DGE so the two streams don't serialize on one sequencer.
            RAMP_K2 = 2
            for k2 in range(RAMP_K2):
                nc.scalar.dma_start(
                    y_sb[:, 2 * k2 : 2 * k2 + 2, :],
                    y_d[k2 * 2 * P : (k2 + 1) * 2 * P, :].rearrange(
                        "(two p) n -> p two n", p=P
                    ),
                )
                adj_sb = adjrpool.tile([P, 2, M], FP8, tag="adjr", name=f"adjr_{k2}")
                for mc in range(MCH):
                    nc.sync.dma_start(
                        adj_sb[:, :, mc * FREE : (mc + 1) * FREE],
                        adjt_d[
                            k2 * 2 * P : (k2 + 1) * 2 * P,
                            mc * FREE : (mc + 1) * FREE,
                        ].rearrange("(two p) m -> p two m", p=P),
                    )
                mm_k2(k2, adj_sb)

            b_sb = bpool.tile([P, NT], F32)
            nc.scalar.dma_start(b_sb[:], b_d[:])

            # Steady phase: 2 k2 tiles (512 contraction rows) per descriptor
            # for both operands; adj alternates between the sync and scalar
            # DGEs to halve per-sequencer issue load and deepen prefetch.
            for q in range(RAMP_K2 // 2, QT):
                nc.scalar.dma_start(
                    y_sb[:, 4 * q : 4 * q + 4, :],
                    y_d[q * 4 * P : (q + 1) * 4 * P, :].rearrange(
                        "(four p) n -> p four n", p=P
                    ),
                )
                adj2_sb = adjpool.tile(
                    [P, 2, 2, M], FP8, tag="adj", name=f"adj_{q}"
                )
                nc.sync.dma_start(
                    adj2_sb[:],
                    adjt_d[q * 4 * P : (q + 1) * 4 * P, :].rearrange(
                        "(kk two p) m -> p kk two m", p=P, two=2
                    ),
                )
                for kk in range(2):
                    mm_k2(2 * q + kk, adj2_sb[:, kk])

            # Epilogue: bias+ReLU straight out of PSUM (ACT and DVE alternate
            # across chunks so the tail isn't serialized on one engine), bf16
            # store, one [128, 512] descriptor per (n, mc) chunk so the last
            # store starts as soon as its own chunk is done.
            for n in range(NT):
                pair_sb = opool.tile([P, 2 * FREE], BF16, tag="osb", name=f"o_{n}")
                for mc in range(MCH):
                    dst = pair_sb[:, mc * FREE : (mc + 1) * FREE]
                    if (2 * n + mc) % 2 == 0:
                        nc.scalar.activation(
                            dst,
                            ps[n][mc][:],
                            mybir.ActivationFunctionType.Relu,
                            bias=b_sb[:, n : n + 1],
                        )
                    else:
                        nc.vector.tensor_scalar(
                            dst,
                            ps[n][mc][:],
                            b_sb[:, n : n + 1],
                            0.0,
                            mybir.AluOpType.add,
                            mybir.AluOpType.max,
                        )
                nc.sync.dma_start(out_d[n * P : (n + 1) * P, :], pair_sb[:])
    nc.finalize()
    return nc


def _get_nc():
    global _NC
    if _NC is None:
        _NC = _build_nc()
    return _NC


def _prepare(inputs):
    f8 = ml_dtypes.float8_e4m3
    x = np.asarray(inputs["x"], dtype=np.float32)
    adj = np.asarray(inputs["adj"], dtype=np.float32)
    W = np.asarray(inputs["W"], dtype=np.float32)
    b = np.asarray(inputs["b"], dtype=np.float32)

    # Host-side algebra: y = x @ W.T exactly; fold adj's mean row into the
    # bias:  adj @ y = (B + 0.5) @ y  =>  b' = b + 0.5 * colsum(y).
    y = x @ W.T
    y8 = np.ascontiguousarray(y.astype(f8))
    bprime = (
        b.astype(np.float64) + 0.5 * y.sum(axis=0, dtype=np.float64)
    ).astype(np.float32)
    b_tiled = np.ascontiguousarray(bprime.reshape(NT, P).T)  # [128, 4]

    b8t = (adj - 0.5).astype(f8).T  # [K, rows] view

    in_maps = []
    for c in range(NCORES):
        in_maps.append(
            {
                "y": y8,
                "adjt": np.ascontiguousarray(b8t[:, c * M : (c + 1) * M]),
                "b": b_tiled,
            }
        )
    return in_maps


def _run(in_maps, **kwargs):
    return run_bass_kernel_spmd(
        _get_nc(), in_maps, core_ids=list(range(NCORES)), **kwargs
    )


def _assemble(results):
    out = np.empty((N_NODES, DIM), dtype=np.float32)
    for c in range(NCORES):
        out[c * M : (c + 1) * M, :] = results[c]["outt"].astype(np.float32).T
    return out


def kernel(**inputs):
    res = _run(_prepare(inputs))
    return _assemble(res.results)
